# revision 3
# baseline (speedup 1.0000x reference)
"""Trainium2 Bass kernel for nn_EncoderText (4-layer SRU text encoder).

Reference computation:
  e = embed[x]                       # [B, T, K]
  4x SRU layers over time (layer0: k=4 projections incl highway; 1-3: k=3)
  gather last valid timestep per sequence, L2-normalize over features.

Strategy:
- Data-parallel over sequences across 8 NeuronCores, no collectives.
- Only VALID tokens are computed: sequences are LPT-bin-packed into
  NCH chunks of CCH columns per core (all cores share one program
  shape; the assignment of sequences to cores/chunks is host-side).
- Activations are channel-major [D, tokens]; the SRU recurrence maps
  onto the DVE tensor_tensor_scan along the free axis with a boundary
  mask zeroing f at sequence starts.
- cand/highway projections run in bf16 (full PE rate, half the weight
  DMA of fp32); the f/r gate projections run in fp8e4 with DoubleRow
  perf mode (2 MACs/cell/cycle) - sigmoid damps the quantization.
- Matmul free dim = full packed token axis (<=512, one PSUM bank).
- Final layer h is DMAed to DRAM in bf16; last-token selection and L2
  normalization happen on the host.
"""

from contextlib import ExitStack

import numpy as np
import ml_dtypes

import concourse.bass as bass
import concourse.mybir as mybir
import concourse.tile as tile
from concourse import bacc
from concourse.bass_utils import run_bass_kernel_spmd
from concourse.masks import make_identity

FP32 = mybir.dt.float32
BF16 = mybir.dt.bfloat16
FP8 = mybir.dt.float8e4
I32 = mybir.dt.int32
AF = mybir.ActivationFunctionType
OP = mybir.AluOpType
DR = mybir.MatmulPerfMode.DoubleRow

NP_BF16 = ml_dtypes.bfloat16
NP_FP8 = ml_dtypes.float8_e4m3

P = 128
N_CORES = 8
FP8_GATES = True


def _ptiles(n):
    """[(start, size)] partition tiles of <=128 covering n."""
    out = []
    s = 0
    while s < n:
        out.append((s, min(P, n - s)))
        s += P
    return out


# ---------------------------------------------------------------------------
# Host-side packing plan
# ---------------------------------------------------------------------------


class Plan:
    def __init__(self, NCH, CCH, SMAX, bins):
        self.NCH = NCH  # chunks per core
        self.CCH = CCH  # columns per chunk
        self.SMAX = SMAX  # max sequences per chunk
        self.bins = bins  # [N_CORES][NCH] -> list of global seq ids
        self.NTOT = NCH * CCH


def make_plan(lengths):
    """LPT bin-pack sequences into N_CORES*NCH bins of capacity CCH."""
    lengths = np.asarray(lengths, np.int64)
    order = np.argsort(-lengths, kind="stable")
    maxlen = int(lengths.max())

    best = None
    for NCH in range(1, 17):
        nbins = N_CORES * NCH
        loads = np.zeros(nbins, np.int64)
        bins = [[] for _ in range(nbins)]
        for i in order:
            j = int(np.argmin(loads))
            loads[j] += int(lengths[i])
            bins[j].append(int(i))
        C = int(loads.max())
        CCH = max(256, maxlen, -(-C // 8) * 8)
        if CCH > 480:
            continue
        cost = NCH * CCH
        if best is None or cost < best[0]:
            SMAX = max(len(b) for b in bins)
            best = (cost, NCH, CCH, SMAX, bins)
        if NCH * 256 >= best[0]:
            break
    assert best is not None, "no feasible packing"
    _, NCH, CCH, SMAX, bins = best
    core_bins = [bins[c * NCH : (c + 1) * NCH] for c in range(N_CORES)]
    return Plan(NCH, CCH, SMAX, core_bins)


def _bands(NTOT, CCH):
    """Token-axis bands of <=512 columns, cut at chunk boundaries."""
    step = max(1, 512 // CCH) * CCH
    if step > 512:
        step = CCH
        assert CCH <= 512, "CCH too large for one PSUM bank"
    return [(s, min(step, NTOT - s)) for s in range(0, NTOT, step)]


# ---------------------------------------------------------------------------
# Device program
# ---------------------------------------------------------------------------


def build_program(K, D, V, NCH, CCH, SMAX):
    """Emit the per-core program.

    DRAM parameters (per core):
      xidx   [NTOKP]      int32  packed token row indices (pad -> 0)
      embed  [V, K]       f32    full embedding table (replicated)
      Wc0 [NKD*2, P, NKB0*P]  bf16  layer0 cand+highway, pre-tiled
      Wg0 [NKD*2, P, KP0*2*P] fp8   layer0 f,r (DoubleRow pairs)
      Wc{1..3} [NKD, P, NKD*P]   bf16  cand
      Wg{1..3} [NKD*2, P, KPD*2*P] fp8 f,r
      bf{l}, br{l} [P, NKD] f32  per-channel-tile bias columns
      bmask  [P, NTOT]    f32    0 at each sequence start (and padding), else 1
      out    [NKD, P, NTOT] bf16 layer-3 hidden, channel-major
    """
    NTOT = NCH * CCH
    NTOKP = ((NTOT + P - 1) // P) * P
    NJ = NTOKP // P
    KT_IN = _ptiles(K)
    KT_D = _ptiles(D)
    NKI, NKD = len(KT_IN), len(KT_D)
    NKB0 = NKI
    KP0 = (NKI + 1) // 2
    KPD = (NKD + 1) // 2
    TOK_T = _ptiles(NTOT)
    BANDS = _bands(NTOT, CCH)

    nc = bacc.Bacc("TRN2", target_bir_lowering=False, debug=False)

    xidx = nc.declare_dram_parameter("xidx", [NTOKP], I32, isOutput=False)
    emb = nc.declare_dram_parameter("embed", [V, K], FP32, isOutput=False)
    Wc_d, Wg_d = [], []
    for l in range(4):
        nkb = NKB0 if l == 0 else NKD
        kp = KP0 if l == 0 else KPD
        ngc = 2 if l == 0 else 1
        Wc_d.append(
            nc.declare_dram_parameter(
                f"Wc{l}", [NKD * ngc, P, nkb * P], BF16, isOutput=False
            )
        )
        if FP8_GATES:
            Wg_d.append(
                nc.declare_dram_parameter(
                    f"Wg{l}", [NKD * 2, P, kp * 2 * P], FP8, isOutput=False
                )
            )
        else:
            Wg_d.append(
                nc.declare_dram_parameter(
                    f"Wg{l}", [NKD * 2, P, nkb * P], BF16, isOutput=False
                )
            )
    bfd = [
        nc.declare_dram_parameter(f"bf{l}", [P, NKD], FP32, isOutput=False)
        for l in range(4)
    ]
    brd = [
        nc.declare_dram_parameter(f"br{l}", [P, NKD], FP32, isOutput=False)
        for l in range(4)
    ]
    bmask_d = nc.declare_dram_parameter("bmask", [P, NTOT], FP32, isOutput=False)
    out_d = nc.declare_dram_parameter("out", [NKD, P, NTOT], BF16, isOutput=True)

    with tile.TileContext(nc) as tc, ExitStack() as ctx:
        sb = ctx.enter_context(tc.tile_pool(name="sb", bufs=1))
        big = ctx.enter_context(tc.tile_pool(name="big", bufs=2))
        wp = ctx.enter_context(tc.tile_pool(name="wp", bufs=1))
        tp = ctx.enter_context(tc.tile_pool(name="tp", bufs=2))
        pp = ctx.enter_context(tc.tile_pool(name="pp", bufs=8, space="PSUM"))

        # ---- constants ----
        identity = sb.tile([P, P], FP32, tag="identity")
        make_identity(nc, identity[:])
        bmask = sb.tile([P, NTOT], FP32, tag="bmask")
        nc.sync.dma_start(out=bmask[:], in_=bmask_d[:, :])
        bfs, brs = [], []
        for l in range(4):
            bft = sb.tile([P, NKD], FP32, tag=f"bf{l}")
            nc.sync.dma_start(out=bft[:], in_=bfd[l][:, :])
            bfs.append(bft)
            brt = sb.tile([P, NKD], FP32, tag=f"br{l}")
            nc.sync.dma_start(out=brt[:], in_=brd[l][:, :])
            brs.append(brt)
        idx_sb = sb.tile([P, NJ], I32, tag="idx")
        nc.sync.dma_start(out=idx_sb[:], in_=xidx[:].rearrange("(j p) -> p j", p=P))

        # ---- embedding gather + transpose to channel-major ----
        eTb = big.tile([P, NKB0, NTOT], BF16, tag="hb", name="eTb")
        if FP8_GATES:
            eT8 = big.tile([P, KP0 * 2, NTOT], FP8, tag="h8", name="eT8")
        # zero the padded channel rows (weights there are zero, but the
        # activations must be finite for 0*x to be 0)
        lastk, lastkk = NKI - 1, KT_IN[-1][1]
        pad0 = (lastkk // 32) * 32  # 32-aligned start; sub-range rewritten below
        if lastkk < P:
            nc.gpsimd.memset(eTb[pad0:P, lastk, :], 0.0)
        if FP8_GATES:
            if lastkk < P:
                nc.gpsimd.memset(eT8[pad0:P, lastk, :], 0.0)
            for kpad in range(NKI, KP0 * 2):
                nc.gpsimd.memset(eT8[:, kpad, :], 0.0)
        for j, (ts_, tj) in enumerate(TOK_T):
            eg = sb.tile([P, K], FP32, tag="eg", bufs=2, name=f"eg{j}")
            nc.gpsimd.indirect_dma_start(
                out=eg[:tj, :],
                out_offset=None,
                in_=emb[:, :],
                in_offset=bass.IndirectOffsetOnAxis(ap=idx_sb[:tj, j : j + 1], axis=0),
            )
            for k, (ks, kk) in enumerate(KT_IN):
                pt = pp.tile([P, 512], FP32, tag="pt", name=f"ptr{j}_{k}")
                nc.tensor.transpose(
                    out=pt[:kk, :tj],
                    in_=eg[:tj, ks : ks + kk],
                    identity=identity[:tj, :tj],
                )
                nc.vector.tensor_copy(
                    out=eTb[:kk, k, ts_ : ts_ + tj], in_=pt[:kk, :tj]
                )
                if FP8_GATES:
                    nc.vector.tensor_copy(
                        out=eT8[:kk, k, ts_ : ts_ + tj], in_=pt[:kk, :tj]
                    )

        # ---- SRU layers ----
        in_b = eTb
        in_8 = eT8 if FP8_GATES else None
        for l in range(4):
            nkb = NKB0 if l == 0 else NKD
            kp = KP0 if l == 0 else KPD

            if l < 3:
                hb_next = big.tile([P, NKD, NTOT], BF16, tag="hb", name=f"hb{l}")
                lk, lkk = NKD - 1, KT_D[-1][1]
                if lkk < P:
                    nc.gpsimd.memset(hb_next[lkk:P, lk, :], 0.0)
                if FP8_GATES:
                    h8_next = big.tile(
                        [P, KPD * 2, NTOT], FP8, tag="h8", name=f"h8{l}"
                    )
                    if lkk < P:
                        nc.gpsimd.memset(h8_next[lkk:P, lk, :], 0.0)
                    for kpad in range(NKD, KPD * 2):
                        nc.gpsimd.memset(h8_next[:, kpad, :], 0.0)

            for ci, (cs, mc) in enumerate(KT_D):
                # weights: cand (+highway for l0) bf16, f/r fp8 DoubleRow
                ngc = 2 if l == 0 else 1
                wc = wp.tile([P, nkb, P], BF16, tag="wc", bufs=2, name=f"wc{l}_{ci}")
                nc.sync.dma_start(
                    out=wc[:, :, :],
                    in_=Wc_d[l][ci * ngc, :, :].rearrange("p (kt m) -> p kt m", m=P),
                )
                if l == 0:
                    wx = wp.tile(
                        [P, nkb, P], BF16, tag="wx", bufs=2, name=f"wx{l}_{ci}"
                    )
                    nc.sync.dma_start(
                        out=wx[:, :, :],
                        in_=Wc_d[l][ci * 2 + 1, :, :].rearrange(
                            "p (kt m) -> p kt m", m=P
                        ),
                    )
                gdt = FP8 if FP8_GATES else BF16
                gk = kp * 2 if FP8_GATES else nkb
                wf = wp.tile([P, gk, P], gdt, tag="wf", bufs=2, name=f"wf{l}_{ci}")
                nc.sync.dma_start(
                    out=wf[:, :, :],
                    in_=Wg_d[l][ci * 2, :, :].rearrange("p (kt m) -> p kt m", m=P),
                )
                wr = wp.tile([P, gk, P], gdt, tag="wr", bufs=2, name=f"wr{l}_{ci}")
                nc.sync.dma_start(
                    out=wr[:, :, :],
                    in_=Wg_d[l][ci * 2 + 1, :, :].rearrange("p (kt m) -> p kt m", m=P),
                )

                if l == 3:
                    hh = tp.tile([P, NTOT], BF16, tag="hh", name=f"hh{ci}")

                for bs, bw in BANDS:
                    bsl = slice(bs, bs + bw)
                    pc = pp.tile([P, 512], FP32, tag="pt", name=f"pc{l}_{ci}_{bs}")
                    for k2 in range(nkb):
                        nc.tensor.matmul(
                            out=pc[:mc, :bw],
                            lhsT=wc[:, k2, :mc],
                            rhs=in_b[:, k2, bsl],
                            start=(k2 == 0),
                            stop=(k2 == nkb - 1),
                        )
                    pf = pp.tile([P, 512], FP32, tag="pt", name=f"pf{l}_{ci}_{bs}")
                    pr = pp.tile([P, 512], FP32, tag="pt", name=f"pr{l}_{ci}_{bs}")
                    if FP8_GATES:
                        for q in range(kp):
                            nc.tensor.matmul(
                                out=pf[:mc, :bw],
                                lhsT=wf[:, 2 * q : 2 * q + 2, :mc],
                                rhs=in_8[:, 2 * q : 2 * q + 2, bsl],
                                start=(q == 0),
                                stop=(q == kp - 1),
                                perf_mode=DR,
                            )
                        for q in range(kp):
                            nc.tensor.matmul(
                                out=pr[:mc, :bw],
                                lhsT=wr[:, 2 * q : 2 * q + 2, :mc],
                                rhs=in_8[:, 2 * q : 2 * q + 2, bsl],
                                start=(q == 0),
                                stop=(q == kp - 1),
                                perf_mode=DR,
                            )
                    else:
                        for k2 in range(nkb):
                            nc.tensor.matmul(
                                out=pf[:mc, :bw],
                                lhsT=wf[:, k2, :mc],
                                rhs=in_b[:, k2, bsl],
                                start=(k2 == 0),
                                stop=(k2 == nkb - 1),
                            )
                        for k2 in range(nkb):
                            nc.tensor.matmul(
                                out=pr[:mc, :bw],
                                lhsT=wr[:, k2, :mc],
                                rhs=in_b[:, k2, bsl],
                                start=(k2 == 0),
                                stop=(k2 == nkb - 1),
                            )
                    if l == 0:
                        px = pp.tile([P, 512], FP32, tag="pt", name=f"px_{ci}_{bs}")
                        for k2 in range(nkb):
                            nc.tensor.matmul(
                                out=px[:mc, :bw],
                                lhsT=wx[:, k2, :mc],
                                rhs=in_b[:, k2, bsl],
                                start=(k2 == 0),
                                stop=(k2 == nkb - 1),
                            )

                    fsb = tp.tile([P, 512], FP32, tag="fsb", name=f"f{l}_{ci}_{bs}")
                    nc.scalar.activation(
                        out=fsb[:mc, :bw],
                        in_=pf[:mc, :bw],
                        func=AF.Sigmoid,
                        bias=bfs[l][:mc, ci : ci + 1],
                    )
                    rsb = tp.tile([P, 512], FP32, tag="rsb", name=f"r{l}_{ci}_{bs}")
                    nc.scalar.activation(
                        out=rsb[:mc, :bw],
                        in_=pr[:mc, :bw],
                        func=AF.Sigmoid,
                        bias=brs[l][:mc, ci : ci + 1],
                    )
                    # z' = (f - 1) * cand  (uses UNMASKED f)
                    zb = tp.tile([P, 512], FP32, tag="zb", name=f"z{l}_{ci}_{bs}")
                    nc.vector.scalar_tensor_tensor(
                        out=zb[:mc, :bw],
                        in0=fsb[:mc, :bw],
                        scalar=1.0,
                        in1=pc[:mc, :bw],
                        op0=OP.subtract,
                        op1=OP.mult,
                    )
                    # masked f on the Pool engine (runs alongside DVE)
                    fm = tp.tile([P, 512], FP32, tag="fm", name=f"fm{l}_{ci}_{bs}")
                    nc.gpsimd.tensor_mul(
                        out=fm[:mc, :bw], in0=fsb[:mc, :bw], in1=bmask[:mc, bsl]
                    )
                    # c_t = fm*c_{t-1} - z'
                    cst = tp.tile([P, 512], FP32, tag="cst", name=f"c{l}_{ci}_{bs}")
                    nc.vector.tensor_tensor_scan(
                        out=cst[:mc, :bw],
                        data0=fm[:mc, :bw],
                        data1=zb[:mc, :bw],
                        initial=0.0,
                        op0=OP.mult,
                        op1=OP.subtract,
                    )
                    # h = r*(tanh(c) - xres) + xres
                    nc.scalar.activation(
                        out=cst[:mc, :bw], in_=cst[:mc, :bw], func=AF.Tanh
                    )
                    if l == 0:
                        xres = px[:mc, :bw]
                    else:
                        xres = in_b[:mc, ci, bsl]
                    nc.vector.tensor_sub(
                        out=cst[:mc, :bw], in0=cst[:mc, :bw], in1=xres
                    )
                    nc.vector.tensor_mul(
                        out=cst[:mc, :bw], in0=cst[:mc, :bw], in1=rsb[:mc, :bw]
                    )
                    if l < 3:
                        nc.vector.tensor_add(
                            out=hb_next[:mc, ci, bsl], in0=cst[:mc, :bw], in1=xres
                        )
                        if FP8_GATES:
                            nc.vector.tensor_add(
                                out=h8_next[:mc, ci, bsl],
                                in0=cst[:mc, :bw],
                                in1=xres,
                            )
                    else:
                        nc.vector.tensor_add(
                            out=hh[:mc, bsl], in0=cst[:mc, :bw], in1=xres
                        )
                if l == 3:
                    nc.sync.dma_start(out=out_d[ci, :mc, :], in_=hh[:mc, :])

            if l < 3:
                in_b = hb_next
                if FP8_GATES:
                    in_8 = h8_next

    nc.compile()
    return nc


# ---------------------------------------------------------------------------
# Host-side input prep
# ---------------------------------------------------------------------------


def _retile(W, Kin, D, gcols, NKD, nkb, np_dt):
    """W [Kin, G*D] -> [NKD*len(gcols), P, nkb*P] pre-tiled for SBUF DMA.

    dest[ci*len+gi, p, kt*P+m] = W[kt*P+p, gcols[gi]*D + ci*P + m]
    """
    ng = len(gcols)
    src = np.asarray(W, np.float32)
    out = np.zeros((NKD * ng, P, nkb * P), np_dt)
    for ci in range(NKD):
        mc = min(P, D - ci * P)
        for gi, g in enumerate(gcols):
            tmp = np.zeros((nkb * P, P), np.float32)
            tmp[:Kin, :mc] = src[:, g * D + ci * P : g * D + ci * P + mc]
            out[ci * ng + gi] = (
                tmp.reshape(nkb, P, P)
                .transpose(1, 0, 2)
                .reshape(P, nkb * P)
                .astype(np_dt)
            )
    return out


def _pack_bias(b_half, D, NKD):
    """[D] -> [128, NKD]: column ci holds channels ci*128..ci*128+127."""
    pad = NKD * P - D
    bp = np.pad(np.asarray(b_half, np.float32), (0, pad))
    return np.ascontiguousarray(bp.reshape(NKD, P).T)


def _retile_weights(Ws, K, D):
    NKD = len(_ptiles(D))
    NKI = len(_ptiles(K))
    KP0 = (NKI + 1) // 2
    KPD = (NKD + 1) // 2
    Wc = [_retile(Ws[0], K, D, [0, 3], NKD, NKI, NP_BF16)]
    if FP8_GATES:
        Wg = [_retile(Ws[0], K, D, [1, 2], NKD, KP0 * 2, NP_FP8)]
    else:
        Wg = [_retile(Ws[0], K, D, [1, 2], NKD, NKI, NP_BF16)]
    for l in range(1, 4):
        Wc.append(_retile(Ws[l], D, D, [0], NKD, NKD, NP_BF16))
        if FP8_GATES:
            Wg.append(_retile(Ws[l], D, D, [1, 2], NKD, KPD * 2, NP_FP8))
        else:
            Wg.append(_retile(Ws[l], D, D, [1, 2], NKD, NKD, NP_BF16))
    return Wc, Wg


def make_core_inputs(core, plan, x, lengths, embed, Wc, Wg, bs, K, D, V):
    NCH, CCH = plan.NCH, plan.CCH
    NTOT = plan.NTOT
    NTOKP = ((NTOT + P - 1) // P) * P
    NKD = len(_ptiles(D))

    xl = np.zeros(NTOKP, np.int32)
    bmask = np.zeros((1, NTOT), np.float32)
    for ch, bin_seqs in enumerate(plan.bins[core]):
        pos = 0
        for b in bin_seqs:
            ln = int(lengths[b])
            if ln <= 0:
                continue
            col0 = ch * CCH + pos
            xl[col0 : col0 + ln] = x[b, :ln]
            bmask[0, col0 + 1 : col0 + ln] = 1.0
            pos += ln

    im = {
        "xidx": xl,
        "embed": np.asarray(embed, np.float32),
        "bmask": np.broadcast_to(bmask, (P, NTOT)).copy(),
    }
    for l in range(4):
        im[f"Wc{l}"] = Wc[l]
        im[f"Wg{l}"] = Wg[l]
        im[f"bf{l}"] = _pack_bias(bs[l][:D], D, NKD)
        im[f"br{l}"] = _pack_bias(bs[l][D:], D, NKD)
    return im


_NC_CACHE = {}


def kernel(x, lengths, embed, W0, b0, W1, b1, W2, b2, W3, b3):
    x = np.asarray(x)
    lengths = np.asarray(lengths)
    embed = np.asarray(embed, np.float32)
    Ws = [np.asarray(w, np.float32) for w in (W0, W1, W2, W3)]
    bs = [np.asarray(b, np.float32) for b in (b0, b1, b2, b3)]

    Bb, T = x.shape
    V, K = embed.shape
    D = Ws[1].shape[0]

    plan = make_plan(lengths)
    key = (K, D, V, plan.NCH, plan.CCH, plan.SMAX)
    if key not in _NC_CACHE:
        _NC_CACHE[key] = build_program(*key)
    nc = _NC_CACHE[key]

    Wc, Wg = _retile_weights(Ws, K, D)
    in_maps = [
        make_core_inputs(c, plan, x, lengths, embed, Wc, Wg, bs, K, D, V)
        for c in range(N_CORES)
    ]
    res = run_bass_kernel_spmd(nc, in_maps, core_ids=list(range(N_CORES)))

    out = np.zeros((Bb, D), np.float32)
    NKD = len(_ptiles(D))
    for c in range(N_CORES):
        oc = np.asarray(res.results[c]["out"], dtype=np.float32)  # [NKD, P, NTOT]
        flat = oc.reshape(NKD * P, plan.NTOT)
        for ch, bin_seqs in enumerate(plan.bins[c]):
            pos = 0
            for b in bin_seqs:
                ln = int(lengths[b])
                if ln <= 0:
                    continue
                col = ch * plan.CCH + pos + ln - 1
                v = flat[:D, col]
                out[b] = v / max(np.linalg.norm(v), 1e-30)
                pos += ln
    return out
